# revision 21
# baseline (speedup 1.0000x reference)
"""Masked phase-locking value (PLV) kernel for Trainium2, 8 NeuronCores.

Math: out[b] = |sum_ij M_ij * exp(i*(a_bi - b_bj))| / max(sum(M), 1)
    real_b = sum_ij M_ij (ca_bi cb_bj + sa_bi sb_bj)
    imag_b = sum_ij M_ij (sa_bi cb_bj - ca_bi sb_bj)

Device decomposition (per core, Na sharded 8 ways -> 1024 mask rows each):
    Z[m, i] = sum_j CST[j, m] * maskT[j, i]      (TensorE, contract full Nb)
      where CST[j, m] = cb[m, j] for m<64, sb[m-64, j] for m>=64  (m = 2B = 128)
    racc[m] = sum_i Z[m, i] * W2[m, i]           (DVE scalar_tensor_tensor,
    qacc[m] = sum_i Z[m, i] * W2S[m, i]           fused mul + row-sum accum)
      W2[m,i]  = ca[m,i] | sa[m-64,i],  W2S[m,i] = sa[m,i] | -ca[m-64,i]
real_b = sum_cores racc[b] + racc[64+b]; imag_b = qacc[b] + qacc[64+b].

Contracting Nb (8192) on the PE and only Na/8 (1024) on the DVE makes the
epilogue 8x smaller than the W-stationary orientation. The matmul runs in
fp8 DoubleRow mode (2 fp8 weights per PE cell, 256-deep contraction per
matmul): 64 matmuls of N=512 -> ~15us of PE time, hidden under the ~22us
of DMA (9.25MB/core at ~420GB/s measured). All DMA rides one HWDGE ring in
dependency order (cst trig first -> it gates the first real matmul; w2/w2s
mid-stream before the first epilogue needs them) so the mask stream never
shares bandwidth at the wrong moment. Mask is 0/1 -> exact in fp8e4; trig
in fp8e4 adds ~2e-3 rel err (the coherent real part accumulates quant
noise as a random walk), inside the 2e-2 gate. A PE warm-up burst on a
memset tile during the DMA lead-in defeats the HAM cold-clock penalty.
"""

import numpy as np

import concourse.bass as bass
import concourse.tile as tile
from concourse import bacc, mybir
from concourse.bass_utils import run_bass_kernel_spmd

B = 64
NA = 8192
NB = 8192
NCORES = 8
NASH = NA // NCORES          # mask rows (i) per core
NBLK = 2                     # i blocks per core (PSUM banks)
IBW = NASH // NBLK           # i columns per block = 512
NJC = NB // 256              # 32 DoubleRow matmuls per block (256-deep each)
JCKP = 2 * NJC               # 64 weight half-chunks of 128 j each
# mask DMA chunk widths per i-block, in jckp units (8 = 0.5MB). Finer final
# chunks shorten the end-of-stream matmul straggle (a chunk gates qw/2 MMs).
CHUNKS = [
    [16, 16, 16, 16],
    [16, 16, 16, 8],          # jckp 0..55 full-width; 56..63 via i-half tail
]
TAILQ = 8                    # jckp in the i-split tail pieces (jc 28..31)
TAILJC = TAILQ // 2
HW = IBW // 2                # 256-wide tail halves

F8 = mybir.dt.float8e4
F16 = mybir.dt.float16
F32 = mybir.dt.float32
DR = mybir.MatmulPerfMode.DoubleRow
MUL = mybir.AluOpType.mult

N_WARM = 8                   # cold-rate N=512 matmuls ~= 3.4us HAM warmup


def build_program() -> bass.Bass:
    nc = bacc.Bacc("TRN2")
    # per-chunk contiguous [p, 16, 512] blocks, chunk-major (ib, q)
    mask_d = nc.dram_tensor("mask", [128 * NBLK * JCKP * IBW], F8, kind="ExternalInput")
    cst_d = nc.dram_tensor("cst", [128 * JCKP * 128], F8, kind="ExternalInput")
    # w2 (cols 0..NASH) and w2s (cols NASH..2*NASH) in one transfer: 2KB per
    # partition keeps the SDMA descriptor spread even (1KB splits straggle)
    w2_d = nc.dram_tensor("w2", [128, 2 * NASH], F8, kind="ExternalInput")
    out_d = nc.dram_tensor("out", [128, 6], F32, kind="ExternalOutput")

    with tile.TileContext(nc) as tc:
        with (
            tc.tile_pool(name="consts", bufs=1) as consts,
            tc.tile_pool(name="scratch", bufs=2) as scratch,
            tc.tile_pool(name="psum", bufs=NBLK, space="PSUM") as psum_pool,
            tc.tile_pool(name="wups", bufs=1, space="PSUM") as wu_pool,
        ):
            # PE warm-up source: memset, no DMA dependency
            wu_in = consts.tile([128, 512], F16)
            nc.vector.memset(wu_in[:], 0.25)

            cst_sb = consts.tile([128, JCKP, 128], F8)
            w2_sb = consts.tile([128, 2 * NASH], F8)
            mt = consts.tile([128, NBLK, JCKP, IBW], F8)
            # last-block tail: jc 28..31 split by i-half, contiguous per half
            mtl = consts.tile([128, 2, TAILQ, HW], F8)

            # one HWDGE ring, dependency order: cst gates the first matmul,
            # w2/w2s slot in before the first epilogue, mask fills the rest
            nc.sync.dma_start(
                out=cst_sb[:],
                in_=cst_d[:].rearrange("(p k m) -> p k m", p=128, k=JCKP),
            )
            off = 0

            def mask_chunk(ib, qs, qw):
                nonlocal off
                blk = 128 * qw * IBW
                nc.sync.dma_start(
                    out=mt[:, ib, qs : qs + qw, :],
                    in_=mask_d[off : off + blk].rearrange(
                        "(p q i) -> p q i", p=128, q=qw
                    ),
                )
                off += blk

            nc.sync.dma_start(out=w2_sb[:], in_=w2_d[:])
            for ib in range(NBLK):
                qs = 0
                for qw in CHUNKS[ib]:
                    mask_chunk(ib, qs, qw)
                    qs += qw
            for h in range(2):
                blk = 128 * TAILQ * HW
                nc.sync.dma_start(
                    out=mtl[:, h, :, :],
                    in_=mask_d[off : off + blk].rearrange(
                        "(p q i) -> p q i", p=128, q=TAILQ
                    ),
                )
                off += blk

            # HAM warm-up while cst + first chunks are in flight
            wu_ps = wu_pool.tile([128, 512], F32)
            for r in range(N_WARM):
                nc.tensor.matmul(
                    out=wu_ps[:],
                    lhsT=wu_in[:, 0:128],
                    rhs=wu_in[:],
                    start=(r == 0),
                    stop=(r == N_WARM - 1),
                )

            # racc cols: 0-2 real (b0, b1h0, b1h1), 3-5 imag
            racc = consts.tile([128, 6], F32)

            def fold(ps_ap, i0, iw, rcol):
                pr = scratch.tile([128, iw], F32, tag="pr")
                nc.vector.scalar_tensor_tensor(
                    out=pr[:], in0=ps_ap, scalar=1.0,
                    in1=w2_sb[:, i0 : i0 + iw],
                    op0=MUL, op1=MUL, accum_out=racc[:, rcol : rcol + 1],
                )
                pi = scratch.tile([128, iw], F32, tag="pr")
                nc.vector.scalar_tensor_tensor(
                    out=pi[:], in0=ps_ap, scalar=1.0,
                    in1=w2_sb[:, NASH + i0 : NASH + i0 + iw],
                    op0=MUL, op1=MUL, accum_out=racc[:, 3 + rcol : 4 + rcol],
                )

            ps0 = psum_pool.tile([128, IBW], F32, tag="psum")
            for jc in range(NJC):
                nc.tensor.matmul(
                    out=ps0[:],
                    lhsT=cst_sb[:, 2 * jc : 2 * jc + 2, :],
                    rhs=mt[:, 0, 2 * jc : 2 * jc + 2, :],
                    start=(jc == 0),
                    stop=(jc == NJC - 1),
                    perf_mode=DR,
                )
            fold(ps0[:], 0, IBW, 0)

            ps1 = psum_pool.tile([128, IBW], F32, tag="psum")
            for jc in range(NJC - TAILJC):
                nc.tensor.matmul(
                    out=ps1[:],
                    lhsT=cst_sb[:, 2 * jc : 2 * jc + 2, :],
                    rhs=mt[:, 1, 2 * jc : 2 * jc + 2, :],
                    start=(jc == 0),
                    stop=False,
                    perf_mode=DR,
                )
            for h in range(2):
                for k in range(TAILJC):
                    jc = NJC - TAILJC + k
                    nc.tensor.matmul(
                        out=ps1[:, h * HW : (h + 1) * HW],
                        lhsT=cst_sb[:, 2 * jc : 2 * jc + 2, :],
                        rhs=mtl[:, h, 2 * k : 2 * k + 2, :],
                        start=False,
                        stop=(k == TAILJC - 1),
                        perf_mode=DR,
                        skip_group_check=True,
                    )
                fold(ps1[:, h * HW : (h + 1) * HW], IBW + h * HW, HW, 1 + h)

            nc.scalar.dma_start(out=out_d[:], in_=racc[:])
    nc.finalize()
    return nc


def prep_inputs(phases_a, phases_b, coupling_mask):
    pa = np.asarray(phases_a, dtype=np.float32)
    pb = np.asarray(phases_b, dtype=np.float32)
    ca, sa = np.cos(pa), np.sin(pa)    # (B, Na)
    cb, sb = np.cos(pb), np.sin(pb)    # (B, Nb)
    f8np = mybir.dt.np(F8)

    # CST[j, m]: cb for m<64, sb for m>=64; tile layout [p, jc, kp, m],
    # j = jc*256 + kp*128 + p
    cst = np.concatenate([cb, sb], axis=0).T.astype(f8np)   # (Nb, 128)
    cst_host = np.ascontiguousarray(
        cst.reshape(NJC, 2, 128, 128).transpose(2, 0, 1, 3)
    ).reshape(-1)

    one_byte = np.array([1.0], f8np).view(np.uint8)[0]
    mask_u8 = (np.asarray(coupling_mask) != 0).astype(np.uint8) * one_byte

    in_maps = []
    for c in range(NCORES):
        rows = slice(c * NASH, (c + 1) * NASH)
        # maskT [j, i] -> [p, ib, jckp, i] -> chunk-major [(ib, q), p, r, i]
        mT = np.ascontiguousarray(mask_u8[rows].T)          # (Nb, NASH)
        A = mT.reshape(NJC, 2, 128, NBLK, IBW).transpose(2, 3, 0, 1, 4)
        pieces = []
        for ib in range(NBLK):
            qs = 0
            for qw in CHUNKS[ib]:
                pieces.append(
                    np.ascontiguousarray(A[:, ib, qs : qs + qw, :]).reshape(-1)
                )
                qs += qw
        for h in range(2):
            pieces.append(
                np.ascontiguousarray(
                    A[:, 1, JCKP - TAILQ :, h * HW : (h + 1) * HW]
                ).reshape(-1)
            )
        m_host = np.concatenate(pieces).view(f8np)

        w2 = np.empty((128, 2 * NASH), np.float32)
        w2[:B, :NASH] = ca[:, rows]
        w2[B:, :NASH] = sa[:, rows]
        w2[:B, NASH:] = sa[:, rows]
        w2[B:, NASH:] = -ca[:, rows]
        in_maps.append(
            {"mask": m_host, "cst": cst_host, "w2": w2.astype(f8np)}
        )
    return in_maps


def combine(outs, coupling_mask):
    o = np.stack(outs).astype(np.float64)   # [NCORES, 128, 6]
    r = o[:, :, :3].sum(axis=2)             # [NCORES, 128]
    q = o[:, :, 3:].sum(axis=2)
    real = (r[:, :B] + r[:, B:]).sum(axis=0)
    imag = (q[:, :B] + q[:, B:]).sum(axis=0)
    n_pairs = max(float(np.asarray(coupling_mask).sum()), 1.0)
    return (np.sqrt(real * real + imag * imag) / n_pairs).astype(np.float32)


_prog_cache: list = []


def kernel(phases_a, phases_b, coupling_mask):
    in_maps = prep_inputs(phases_a, phases_b, coupling_mask)
    if not _prog_cache:
        _prog_cache.append(build_program())
    res = run_bass_kernel_spmd(_prog_cache[0], in_maps, core_ids=list(range(NCORES)))
    return combine([r["out"] for r in res.results], coupling_mask)
